# revision 7
# baseline (speedup 1.0000x reference)
"""Trainium2 Bass kernel for nn_LocalOptLoss (batch 16384, data-parallel on 8 cores).

v4: 4-lane block-diagonal packing (2048 samples/core = 4 lanes x 512 cols), so
every per-sample matvec (K,M <= 32) is ONE matmul instruction using the full
128x128 PE array. Matvec chains with no nonlinearity between them are folded
into host-precomputed weight products, which removes all intermediate
PSUM->SBUF copies. Inputs arrive in 3 parallel DMAs (x+e merged, biases
bitcast into the critical weight pack); warmup matmuls on a zeroed tile run
during the DMA window so the PE HAM clock-gate opens before the real chain.
The per-sample residual vector v is DMA'd out raw; the host does norm + mean.
"""
import sys

sys.path.insert(0, "/opt/trn_rl_repo")

from contextlib import ExitStack

import numpy as np
from ml_dtypes import bfloat16 as np_bf16

import concourse.bass as bass
import concourse.bacc as bacc
import concourse.tile as tile
from concourse import mybir
from concourse.bass_utils import run_bass_kernel_spmd

N, NZ, OUT, H, B = 16, 16, 8, 32, 16384
R = 0.1
NCORES = 8
PER_CORE = B // NCORES          # 2048
LANES = 4
COLS = PER_CORE // LANES        # 512
F = COLS

F32 = mybir.dt.float32
BF16 = mybir.dt.bfloat16
AF = mybir.ActivationFunctionType
ALU = mybir.AluOpType

# ---- packed weight layout (block-diagonal lhsT tiles) ----
# biases live in pack0 cols 0..11 as bitcast fp32 pairs; weights follow.
BIAS_NAMES = ["bT1", "btau1", "bh1", "bpsi1", "bf1", "zero"]
NBC = 2 * len(BIAS_NAMES)       # bf16 cols holding fp32 biases
_WL = {}
_PC = [NBC, 0]                  # col cursors for pack0 / pack1


def _wadd(pack, name, in_p, out_p, r0=0):
    _WL[name] = (pack, in_p, out_p, _PC[pack], r0)
    _PC[pack] += LANES * out_p


# w_tau1/w_q3 consume e (SBUF partitions 64-127), so their lhsT blocks sit at
# pack rows 64-127 to satisfy the matmul base-partition match.
_wadd(0, "w_pre1", 16, 32)      # WT1
_wadd(0, "w_pre2a", 32, 32)     # Wtau1 @ WT2   (also pt2)
_wadd(0, "w_tau1", 16, 32, 64)  # Wtau1          (pre2b, q1)
_wadd(1, "w_th", 32, 32)        # Wh1 @ Wtau2
_wadd(1, "w_hd1", 16, 32)       # Wh1
_wadd(1, "w_hd2n", 32, 32)      # -(Wh1 @ Wtau2)
_wadd(1, "w_a3", 32, 32)        # WT1 @ Wtau2
_wadd(1, "w_ppsi1", 32, 32)     # Wpsi1z @ WT2
_wadd(1, "w_ppsi2", 32, 32)     # Wpsi1y @ Wh2
_wadd(1, "w_r", 32, 32)         # Wtau1 @ Wpsi2  (r, q4)
_wadd(1, "w_bk", 32, 32)        # Wh2.T @ Wh2 / R
_wadd(1, "w_glin", 32, 16)      # Wh1 (as lhsT for Wh1.T @ .)
_wadd(1, "w_ones", 16, 16)      # ones(16,16)
_wadd(1, "w_s", 32, 16)         # WP @ Wtau2
_wadd(1, "w_q3", 16, 32, 64)        # Wpsi1z
_wadd(1, "w_tf", 32, 32)        # Wf1 @ Wtau2    (tfpre, q2)
_wadd(1, "w_f2p", 32, 16)       # Wf2
_wadd(1, "w_tau2n", 32, 16)     # -Wtau2
WCOLS0, WCOLS1 = _PC
NWARM = 3                       # HAM warmup matmuls during the DMA window


def build_nc():
    nc = bacc.Bacc("TRN2", target_bir_lowering=False, debug=False,
                   num_devices=NCORES)
    xe_d = nc.dram_tensor("xe", [128, COLS], BF16, kind="ExternalInput")
    w0_d = nc.dram_tensor("wcrit", [128, WCOLS0], BF16, kind="ExternalInput")
    w1_d = nc.dram_tensor("wrest", [128, WCOLS1], BF16, kind="ExternalInput")
    out_d = nc.dram_tensor("vout", [4 * N, COLS], BF16, kind="ExternalOutput")

    with tile.TileContext(nc) as tc, ExitStack() as ctx:
        wt = ctx.enter_context(tc.tile_pool(name="wt", bufs=1))
        acts = ctx.enter_context(tc.tile_pool(name="acts", bufs=1))
        ps = ctx.enter_context(tc.tile_pool(name="ps", bufs=8, space="PSUM"))

        w0 = wt.tile([128, WCOLS0], BF16, tag="w0", name="w0")
        w1 = wt.tile([128, WCOLS1], BF16, tag="w1", name="w1")
        packs = {0: w0, 1: w1}
        w = {}
        for k, (p, in_p, out_p, c0, r0) in _WL.items():
            w[k] = packs[p][r0:r0 + LANES * in_p, c0:c0 + LANES * out_p]
        bias = {n: w0[:, 2 * i:2 * i + 2].bitcast(F32)
                for i, n in enumerate(BIAS_NAMES)}

        xe = wt.tile([128, COLS], BF16, tag="xe", name="xe")
        x_t, e_t = xe[0:64], xe[64:128]
        vall = wt.tile([4 * N, COLS], BF16, tag="vall")
        junk = wt.tile([128, F], BF16, tag="junk", name="junk")

        # Parallel input DMAs on separate queues + PE warmup during the wait.
        nc.sync.dma_start(out=w0, in_=w0_d.ap())
        nc.gpsimd.memset(junk, 0.0)
        nc.gpsimd.dma_start(out=xe, in_=xe_d.ap())
        nc.sync.dma_start(out=w1, in_=w1_d.ap())

        # Warm the Tanh table early; reading w0 also advances the scalar
        # engine's clock past the critical-weight DMA.
        dummy = wt.tile([1, 1], BF16, tag="dummy", name="dummy")
        nc.scalar.activation(dummy, w0[0:1, 0:1], AF.Tanh,
                             bias=bias["zero"][0:1])

        def mm(out, lhsT, rhs, start=True, stop=True):
            nc.tensor.matmul(out, lhsT, rhs, start=start, stop=stop)

        p_junk = ps.tile([128, F], F32, tag="ps", name="p_junk")
        for _ in range(NWARM):
            mm(p_junk, junk[:, 0:128], junk)

        # ---- forward chain (critical-path-first issue order) ----
        p_pre1 = ps.tile([128, F], F32, tag="ps")
        mm(p_pre1, w["w_pre1"], x_t)
        a1 = acts.tile([128, F], BF16, tag="a1")
        nc.scalar.activation(a1, p_pre1, AF.Tanh, bias=bias["bT1"])

        p_pre2 = ps.tile([128, F], F32, tag="ps")
        mm(p_pre2, w["w_pre2a"], a1, start=True, stop=False)
        mm(p_pre2, w["w_tau1"], e_t, start=False, stop=True)
        a2 = acts.tile([128, F], BF16, tag="a2")
        nc.scalar.activation(a2, p_pre2, AF.Tanh, bias=bias["btau1"])

        # term1 head: th -> sq_th (scalar-queue-local) -> argh1
        p_th = ps.tile([128, F], F32, tag="ps")
        mm(p_th, w["w_th"], a2)
        th = acts.tile([128, F], BF16, tag="th")
        nc.scalar.activation(th, p_th, AF.Tanh, bias=bias["bh1"])
        sq_th = acts.tile([128, F], BF16, tag="sq_th")
        nc.scalar.activation(sq_th, th, AF.Square, bias=bias["zero"])

        p_hd = ps.tile([128, F], F32, tag="ps")
        mm(p_hd, w["w_hd1"], x_t, start=True, stop=False)
        mm(p_hd, w["w_hd2n"], a2, start=False, stop=True)
        argh1 = acts.tile([128, F], BF16, tag="argh1")
        nc.vector.scalar_tensor_tensor(argh1, sq_th, -1.0, p_hd,
                                       ALU.add, ALU.mult)

        # he-chain front: a3 -> (ppsi, pt2) -> (tp, t2)
        p_a3 = ps.tile([128, F], F32, tag="ps")
        mm(p_a3, w["w_a3"], a2)
        a3 = acts.tile([128, F], BF16, tag="a3")
        nc.scalar.activation(a3, p_a3, AF.Tanh, bias=bias["bT1"])

        p_ppsi = ps.tile([128, F], F32, tag="ps")
        mm(p_ppsi, w["w_ppsi1"], a3, start=True, stop=False)
        mm(p_ppsi, w["w_ppsi2"], th, start=False, stop=True)
        tp = acts.tile([128, F], BF16, tag="tp")
        nc.scalar.activation(tp, p_ppsi, AF.Tanh, bias=bias["bpsi1"])

        p_t2 = ps.tile([128, F], F32, tag="ps")
        mm(p_t2, w["w_pre2a"], a3)
        t2 = acts.tile([128, F], BF16, tag="t2")
        nc.scalar.activation(t2, p_t2, AF.Tanh, bias=bias["btau1"])
        sq_t2 = acts.tile([128, F], BF16, tag="sq_t2")
        nc.vector.tensor_mul(sq_t2, t2, t2)

        p_q1 = ps.tile([128, F], F32, tag="ps")
        mm(p_q1, w["w_tau1"], e_t)
        argA2n = acts.tile([128, F], BF16, tag="argA2n")
        nc.vector.scalar_tensor_tensor(argA2n, sq_t2, -1.0, p_q1,
                                       ALU.add, ALU.mult)
        m2 = acts.tile([128, F], BF16, tag="m2")
        nc.vector.scalar_tensor_tensor(m2, t2, 2.0, argA2n,
                                       ALU.mult, ALU.mult)

        p_r = ps.tile([128, F], F32, tag="ps")
        mm(p_r, w["w_r"], tp)
        argH = acts.tile([128, F], BF16, tag="argH")
        nc.vector.tensor_mul(argH, m2, p_r)

        # term1 tail
        p_bk = ps.tile([128, F], F32, tag="ps")
        mm(p_bk, w["w_bk"], argh1)
        argh2 = acts.tile([128, F], BF16, tag="argh2")
        nc.vector.scalar_tensor_tensor(argh2, sq_th, -1.0, p_bk,
                                       ALU.add, ALU.mult)

        p_ga = ps.tile([128, F], F32, tag="ps")   # glin alone, for s.(s^T glin)
        mm(p_ga[0:64], w["w_glin"], argh2)
        p_b = ps.tile([128, F], F32, tag="ps")    # glin - term2e accumulator
        mm(p_b[0:64], w["w_glin"], argh2, start=True, stop=False)

        p_s = ps.tile([128, F], F32, tag="ps")
        mm(p_s[0:64], w["w_s"], a2)
        s = acts.tile([64, F], BF16, tag="s")
        nc.scalar.activation(s, p_s[0:64], AF.Tanh, bias=bias["zero"][0:64])

        m = acts.tile([64, F], BF16, tag="m")
        nc.vector.tensor_mul(m, s, p_ga[0:64])
        p_d = ps.tile([128, F], F32, tag="ps")
        mm(p_d[0:64], w["w_ones"], m)
        sm = acts.tile([64, F], BF16, tag="sm")
        nc.vector.tensor_mul(sm, s, p_d[0:64])

        # jp-chain: q3 -> argA1 -> q4 -> argP
        p_q3 = ps.tile([128, F], F32, tag="ps")
        mm(p_q3, w["w_q3"], e_t)
        sq_tp = acts.tile([128, F], BF16, tag="sq_tp")
        nc.gpsimd.tensor_mul(sq_tp, tp, tp)
        argA1 = acts.tile([128, F], BF16, tag="argA1")
        nc.vector.scalar_tensor_tensor(argA1, sq_tp, -1.0, p_q3,
                                       ALU.add, ALU.mult)
        p_q4 = ps.tile([128, F], F32, tag="ps")
        mm(p_q4, w["w_r"], argA1)
        argP = acts.tile([128, F], BF16, tag="argP")
        nc.vector.scalar_tensor_tensor(argP, sq_t2, -1.0, p_q4,
                                       ALU.add, ALU.mult)

        # fu-chain: tf -> sq_tf -> q2 -> argF
        p_tf = ps.tile([128, F], F32, tag="ps")
        mm(p_tf, w["w_tf"], a2)
        tf = acts.tile([128, F], BF16, tag="tf")
        nc.scalar.activation(tf, p_tf, AF.Tanh, bias=bias["bf1"])
        sq_tf = acts.tile([128, F], BF16, tag="sq_tf")
        nc.gpsimd.tensor_mul(sq_tf, tf, tf)
        p_q2 = ps.tile([128, F], F32, tag="ps")
        mm(p_q2, w["w_tf"], argA2n)
        argF = acts.tile([128, F], BF16, tag="argF")
        nc.vector.scalar_tensor_tensor(argF, sq_tf, -1.0, p_q2,
                                       ALU.add, ALU.mult)

        # B accumulation: + Wf2 argF - Wtau2 argP - Wtau2 argH
        mm(p_b[0:64], w["w_f2p"], argF, start=False, stop=False)
        mm(p_b[0:64], w["w_tau2n"], argP, start=False, stop=False)
        mm(p_b[0:64], w["w_tau2n"], argH, start=False, stop=True)

        nc.vector.tensor_add(vall, sm, p_b[0:64])
        nc.sync.dma_start(out=out_d.ap(), in_=vall)

    nc.compile()
    return nc


def _host_weights(Wf1, bf1, Wf2, Wh1, bh1, Wh2, WT1, bT1, WT2,
                  Wtau1, btau1, Wtau2, Wpsi1, bpsi1, Wpsi2, WP):
    f = np.float64
    A = lambda a: np.asarray(a, f)
    Wf1, Wf2, Wh1, Wh2 = A(Wf1), A(Wf2), A(Wh1), A(Wh2)
    WT1, WT2, Wtau1, Wtau2 = A(WT1), A(WT2), A(Wtau1), A(Wtau2)
    Wpsi1, Wpsi2, WP = A(Wpsi1), A(Wpsi2), A(WP)
    Wpsi1z, Wpsi1y = Wpsi1[:, :NZ], Wpsi1[:, NZ:]

    vals = {
        "w_pre1": WT1, "w_pre2a": Wtau1 @ WT2, "w_tau1": Wtau1,
        "w_a3": WT1 @ Wtau2, "w_s": WP @ Wtau2, "w_tf": Wf1 @ Wtau2,
        "w_th": Wh1 @ Wtau2, "w_hd1": Wh1, "w_hd2n": -(Wh1 @ Wtau2),
        "w_ppsi1": Wpsi1z @ WT2, "w_ppsi2": Wpsi1y @ Wh2,
        "w_q3": Wpsi1z, "w_r": Wtau1 @ Wpsi2, "w_bk": Wh2.T @ Wh2 / R,
        "w_glin": Wh1.T, "w_f2p": Wf2, "w_tau2n": -Wtau2,
        "w_ones": np.ones((16, 16), f),
    }
    packs = {0: np.zeros((128, WCOLS0), np.float32),
             1: np.zeros((128, WCOLS1), np.float32)}
    for k, (p, in_p, out_p, c0, r0) in _WL.items():
        WT = vals[k].T          # per-lane lhsT block (in_p, out_p)
        assert WT.shape == (in_p, out_p), (k, WT.shape, (in_p, out_p))
        for L in range(LANES):
            packs[p][r0 + L * in_p:r0 + (L + 1) * in_p,
                     c0 + L * out_p:c0 + (L + 1) * out_p] = WT
    p0 = packs[0].astype(np_bf16)
    p1 = packs[1].astype(np_bf16)
    # biases: fp32 values bitcast into bf16 column pairs of pack0
    bvals = {"bT1": bT1, "btau1": btau1, "bh1": bh1, "bpsi1": bpsi1,
             "bf1": bf1, "zero": np.zeros(H, np.float32)}
    for i, n in enumerate(BIAS_NAMES):
        col = np.tile(np.asarray(bvals[n], np.float32), LANES).reshape(128, 1)
        p0[:, 2 * i:2 * i + 2] = col.view(np_bf16)
    return {"wcrit": p0, "wrest": p1}


_CACHE = {}


def _get_nc():
    if "nc" not in _CACHE:
        _CACHE["nc"] = build_nc()
    return _CACHE["nc"]


def _in_maps(x_batch, e_batch, wts):
    wmap = _host_weights(**wts)

    def lanes(a, rows):
        return (np.asarray(a, np.float32).reshape(LANES, COLS, rows)
                .transpose(0, 2, 1).reshape(LANES * rows, COLS))

    in_maps = []
    for c in range(NCORES):
        cs = slice(c * PER_CORE, (c + 1) * PER_CORE)
        xe = np.concatenate([lanes(x_batch[cs], N), lanes(e_batch[cs], NZ)],
                            axis=0)
        m = {"xe": np.ascontiguousarray(xe.astype(np_bf16))}
        m.update(wmap)
        in_maps.append(m)
    return in_maps


def _reduce(results):
    total = np.float64(0.0)
    for r in results:
        v = np.asarray(r["vout"], np.float64)      # (64, COLS)
        v = v.reshape(LANES, N, COLS)              # lane, feature, col
        total += np.sqrt((v * v).sum(axis=1)).sum()
    return np.asarray(total / B, dtype=np.float32)


def kernel(x_batch, e_batch, **wts):
    nc = _get_nc()
    in_maps = _in_maps(np.asarray(x_batch, np.float32),
                       np.asarray(e_batch, np.float32), wts)
    res = run_bass_kernel_spmd(nc, in_maps, core_ids=list(range(NCORES)))
    return _reduce(res.results)


if __name__ == "__main__":
    rng = np.random.default_rng(0)
    wts = {
        "Wf1": rng.normal(size=(H, N)) * .3, "bf1": rng.normal(size=(H,)) * .3,
        "Wf2": rng.normal(size=(N, H)) * .3,
        "Wh1": rng.normal(size=(H, N)) * .3, "bh1": rng.normal(size=(H,)) * .3,
        "Wh2": rng.normal(size=(OUT, H)) * .3,
        "WT1": rng.normal(size=(H, N)) * .3, "bT1": rng.normal(size=(H,)) * .3,
        "WT2": rng.normal(size=(NZ, H)) * .3,
        "Wtau1": rng.normal(size=(H, NZ)) * .3, "btau1": rng.normal(size=(H,)) * .3,
        "Wtau2": rng.normal(size=(N, H)) * .3,
        "Wpsi1": rng.normal(size=(H, NZ + OUT)) * .3, "bpsi1": rng.normal(size=(H,)) * .3,
        "Wpsi2": rng.normal(size=(NZ, H)) * .3,
        "WP": rng.normal(size=(N, N)) * .3,
    }
    x = rng.normal(size=(B, N)).astype(np.float32)
    e = (rng.normal(size=(B, NZ)) * 0.1).astype(np.float32)
    print(kernel(x, e, **{k: np.asarray(v, np.float32) for k, v in wts.items()}))


# revision 9
# speedup vs baseline: 1.1277x; 1.1277x over previous
"""Trainium2 Bass kernel for nn_LocalOptLoss (batch 16384, data-parallel on 8 cores).

v4: 4-lane block-diagonal packing (2048 samples/core = 4 lanes x 512 cols), so
every per-sample matvec (K,M <= 32) is ONE matmul instruction using the full
128x128 PE array. Matvec chains with no nonlinearity between them are folded
into host-precomputed weight products, which removes all intermediate
PSUM->SBUF copies. Inputs arrive in 3 parallel DMAs (x+e merged, biases
bitcast into the critical weight pack); warmup matmuls on a zeroed tile run
during the DMA window so the PE HAM clock-gate opens before the real chain.
The per-sample residual vector v is DMA'd out raw; the host does norm + mean.
"""
import sys

sys.path.insert(0, "/opt/trn_rl_repo")

from contextlib import ExitStack

import numpy as np
from ml_dtypes import bfloat16 as np_bf16

import concourse.bass as bass
import concourse.bacc as bacc
import concourse.tile as tile
from concourse import mybir
from concourse.bass_utils import run_bass_kernel_spmd

N, NZ, OUT, H, B = 16, 16, 8, 32, 16384
R = 0.1
NCORES = 8
PER_CORE = B // NCORES          # 2048
LANES = 4
COLS = PER_CORE // LANES        # 512
F = COLS

F32 = mybir.dt.float32
BF16 = mybir.dt.bfloat16
AF = mybir.ActivationFunctionType
ALU = mybir.AluOpType

# ---- packed weight layout (block-diagonal lhsT tiles) ----
# biases live in pack0 cols 0..11 as bitcast fp32 pairs; weights follow.
BIAS_NAMES = ["bT1", "btau1", "bh1", "bpsi1", "bf1", "zero"]
NBC = 2 * len(BIAS_NAMES)       # bf16 cols holding fp32 biases
_WL = {}
_PC = [NBC, 0]                  # col cursors for pack0 / pack1


def _wadd(pack, name, in_p, out_p, r0=0):
    _WL[name] = (pack, in_p, out_p, _PC[pack], r0)
    _PC[pack] += LANES * out_p


# w_tau1/w_q3 consume e (SBUF partitions 64-127), so their lhsT blocks sit at
# pack rows 64-127 to satisfy the matmul base-partition match.
_wadd(0, "w_pre1", 16, 32)      # WT1
_wadd(0, "w_pre2a", 32, 32)     # Wtau1 @ WT2   (also pt2)
_wadd(0, "w_tau1", 16, 32, 64)  # Wtau1          (pre2b, q1)
_wadd(1, "w_th", 32, 32)        # Wh1 @ Wtau2
_wadd(1, "w_hd1", 16, 32)       # Wh1
_wadd(1, "w_hd2n", 32, 32)      # -(Wh1 @ Wtau2)
_wadd(1, "w_a3", 32, 32)        # WT1 @ Wtau2
_wadd(1, "w_ppsi1", 32, 32)     # Wpsi1z @ WT2
_wadd(1, "w_ppsi2", 32, 32)     # Wpsi1y @ Wh2
_wadd(1, "w_r", 32, 32)         # Wtau1 @ Wpsi2  (r, q4)
_wadd(1, "w_bk", 32, 32)        # Wh2.T @ Wh2 / R
_wadd(1, "w_glin", 32, 16)      # Wh1 (as lhsT for Wh1.T @ .)
_wadd(1, "w_ones", 16, 16)      # ones(16,16)
_wadd(1, "w_s", 32, 16)         # WP @ Wtau2
_wadd(1, "w_q3", 16, 32, 64)        # Wpsi1z
_wadd(1, "w_tf", 32, 32)        # Wf1 @ Wtau2    (tfpre, q2)
_wadd(1, "w_f2p", 32, 16)       # Wf2
_wadd(1, "w_tau2n", 32, 16)     # -Wtau2
WCOLS0, WCOLS1 = _PC
NWARM = 3                       # HAM warmup matmuls during the DMA window


def build_nc():
    nc = bacc.Bacc("TRN2", target_bir_lowering=False, debug=False,
                   num_devices=NCORES)
    xe_d = nc.dram_tensor("xe", [128, COLS], BF16, kind="ExternalInput")
    w0_d = nc.dram_tensor("wcrit", [128, WCOLS0], BF16, kind="ExternalInput")
    w1_d = nc.dram_tensor("wrest", [128, WCOLS1], BF16, kind="ExternalInput")
    out_d = nc.dram_tensor("vout", [4 * N, COLS], BF16, kind="ExternalOutput")

    with tile.TileContext(nc) as tc, ExitStack() as ctx:
        wt = ctx.enter_context(tc.tile_pool(name="wt", bufs=1))
        acts = ctx.enter_context(tc.tile_pool(name="acts", bufs=1))
        ps = ctx.enter_context(tc.tile_pool(name="ps", bufs=7, space="PSUM"))

        w0 = wt.tile([128, WCOLS0], BF16, tag="w0", name="w0")
        w1 = wt.tile([128, WCOLS1], BF16, tag="w1", name="w1")
        packs = {0: w0, 1: w1}
        w = {}
        for k, (p, in_p, out_p, c0, r0) in _WL.items():
            w[k] = packs[p][r0:r0 + LANES * in_p, c0:c0 + LANES * out_p]
        bias = {n: w0[:, 2 * i:2 * i + 2].bitcast(F32)
                for i, n in enumerate(BIAS_NAMES)}

        xe = wt.tile([128, COLS], BF16, tag="xe", name="xe")
        x_t, e_t = xe[0:64], xe[64:128]
        vall = wt.tile([4 * N, COLS], BF16, tag="vall")

        # Parallel input DMAs on separate queues.
        nc.sync.dma_start(out=w0, in_=w0_d.ap())
        nc.gpsimd.dma_start(out=xe, in_=xe_d.ap())
        nc.sync.dma_start(out=w1, in_=w1_d.ap())

        # Warm the Tanh table early; reading w0 also advances the scalar
        # engine's clock past the critical-weight DMA.
        dummy = wt.tile([1, 1], BF16, tag="dummy", name="dummy")
        nc.scalar.activation(dummy, w0[0:1, 0:1], AF.Tanh,
                             bias=bias["zero"][0:1])

        def mm(out, lhsT, rhs, start=True, stop=True):
            nc.tensor.matmul(out, lhsT, rhs, start=start, stop=stop)

        # Filler matmuls re-run pre1 into a junk bank: dependencies are
        # already satisfied, so they soak up tensor-queue stalls and keep the
        # PE HAM clock-gate open (cold MMs run at half clock otherwise).
        p_junk = ps.tile([128, F], F32, tag="psj", bufs=1, name="p_junk")

        def fill():
            mm(p_junk, w["w_pre1"], x_t)

        # ---- forward chain ----
        p_pre1 = ps.tile([128, F], F32, tag="ps")
        mm(p_pre1, w["w_pre1"], x_t)
        a1 = acts.tile([128, F], BF16, tag="a1")
        nc.scalar.activation(a1, p_pre1, AF.Tanh, bias=bias["bT1"])
        fill()
        fill()

        p_pre2 = ps.tile([128, F], F32, tag="ps")
        mm(p_pre2, w["w_pre2a"], a1, start=True, stop=False)
        mm(p_pre2, w["w_tau1"], e_t, start=False, stop=True)
        a2 = acts.tile([128, F], BF16, tag="a2")
        nc.scalar.activation(a2, p_pre2, AF.Tanh, bias=bias["btau1"])
        fill()
        fill()

        # Dense burst of a2-consumers keeps the PE stream gap-free.
        p_th = ps.tile([128, F], F32, tag="ps")
        mm(p_th, w["w_th"], a2)
        p_hd = ps.tile([128, F], F32, tag="ps")
        mm(p_hd, w["w_hd1"], x_t, start=True, stop=False)
        mm(p_hd, w["w_hd2n"], a2, start=False, stop=True)
        p_a3 = ps.tile([128, F], F32, tag="ps")
        mm(p_a3, w["w_a3"], a2)
        p_tf = ps.tile([128, F], F32, tag="ps")
        mm(p_tf, w["w_tf"], a2)
        p_s = ps.tile([128, F], F32, tag="ps")
        mm(p_s[0:64], w["w_s"], a2)

        th = acts.tile([128, F], BF16, tag="th")
        nc.scalar.activation(th, p_th, AF.Tanh, bias=bias["bh1"])
        a3 = acts.tile([128, F], BF16, tag="a3")
        nc.scalar.activation(a3, p_a3, AF.Tanh, bias=bias["bT1"])
        sq_th = acts.tile([128, F], BF16, tag="sq_th")
        nc.scalar.activation(sq_th, th, AF.Square, bias=bias["zero"])
        argh1 = acts.tile([128, F], BF16, tag="argh1")
        nc.vector.scalar_tensor_tensor(argh1, sq_th, -1.0, p_hd,
                                       ALU.add, ALU.mult)

        # second level: ppsi/pt2 off a3+th, bk off argh1, q1/q3 off e
        p_ppsi = ps.tile([128, F], F32, tag="ps")
        mm(p_ppsi, w["w_ppsi1"], a3, start=True, stop=False)
        mm(p_ppsi, w["w_ppsi2"], th, start=False, stop=True)
        p_t2 = ps.tile([128, F], F32, tag="ps")
        mm(p_t2, w["w_pre2a"], a3)
        p_bk = ps.tile([128, F], F32, tag="ps")
        mm(p_bk, w["w_bk"], argh1)
        p_q1 = ps.tile([128, F], F32, tag="ps")
        mm(p_q1, w["w_tau1"], e_t)

        tp = acts.tile([128, F], BF16, tag="tp")
        nc.scalar.activation(tp, p_ppsi, AF.Tanh, bias=bias["bpsi1"])
        t2 = acts.tile([128, F], BF16, tag="t2")
        nc.scalar.activation(t2, p_t2, AF.Tanh, bias=bias["btau1"])
        sq_t2 = acts.tile([128, F], BF16, tag="sq_t2")
        nc.vector.tensor_mul(sq_t2, t2, t2)
        argA2n = acts.tile([128, F], BF16, tag="argA2n")
        nc.vector.scalar_tensor_tensor(argA2n, sq_t2, -1.0, p_q1,
                                       ALU.add, ALU.mult)
        m2 = acts.tile([128, F], BF16, tag="m2")
        nc.vector.scalar_tensor_tensor(m2, t2, 2.0, argA2n,
                                       ALU.mult, ALU.mult)
        argh2 = acts.tile([128, F], BF16, tag="argh2")
        nc.vector.scalar_tensor_tensor(argh2, sq_th, -1.0, p_bk,
                                       ALU.add, ALU.mult)

        p_r = ps.tile([128, F], F32, tag="ps")
        mm(p_r, w["w_r"], tp)
        p_ga = ps.tile([128, F], F32, tag="ps")
        mm(p_ga[0:64], w["w_glin"], argh2)
        p_b = ps.tile([128, F], F32, tag="ps")
        mm(p_b[0:64], w["w_glin"], argh2, start=True, stop=False)
        p_q3 = ps.tile([128, F], F32, tag="ps")
        mm(p_q3, w["w_q3"], e_t)
        p_q2 = ps.tile([128, F], F32, tag="ps")
        mm(p_q2, w["w_tf"], argA2n)

        s = acts.tile([64, F], BF16, tag="s")
        nc.scalar.activation(s, p_s[0:64], AF.Tanh, bias=bias["zero"][0:64])
        tf = acts.tile([128, F], BF16, tag="tf")
        nc.scalar.activation(tf, p_tf, AF.Tanh, bias=bias["bf1"])
        sq_tf = acts.tile([128, F], BF16, tag="sq_tf")
        nc.gpsimd.tensor_mul(sq_tf, tf, tf)
        sq_tp = acts.tile([128, F], BF16, tag="sq_tp")
        nc.gpsimd.tensor_mul(sq_tp, tp, tp)

        argH = acts.tile([128, F], BF16, tag="argH")
        nc.vector.tensor_mul(argH, m2, p_r)
        m = acts.tile([64, F], BF16, tag="m")
        nc.vector.tensor_mul(m, s, p_ga[0:64])
        argA1 = acts.tile([128, F], BF16, tag="argA1")
        nc.vector.scalar_tensor_tensor(argA1, sq_tp, -1.0, p_q3,
                                       ALU.add, ALU.mult)
        argF = acts.tile([128, F], BF16, tag="argF")
        nc.vector.scalar_tensor_tensor(argF, sq_tf, -1.0, p_q2,
                                       ALU.add, ALU.mult)

        p_d = ps.tile([128, F], F32, tag="ps")
        mm(p_d[0:64], w["w_ones"], m)
        p_q4 = ps.tile([128, F], F32, tag="ps")
        mm(p_q4, w["w_r"], argA1)
        sm = acts.tile([64, F], BF16, tag="sm")
        nc.vector.tensor_mul(sm, s, p_d[0:64])
        argP = acts.tile([128, F], BF16, tag="argP")
        nc.vector.scalar_tensor_tensor(argP, sq_t2, -1.0, p_q4,
                                       ALU.add, ALU.mult)

        # B accumulation: + Wf2 argF - Wtau2 argP - Wtau2 argH
        mm(p_b[0:64], w["w_f2p"], argF, start=False, stop=False)
        mm(p_b[0:64], w["w_tau2n"], argH, start=False, stop=False)
        mm(p_b[0:64], w["w_tau2n"], argP, start=False, stop=True)

        nc.vector.tensor_add(vall, sm, p_b[0:64])
        nc.sync.dma_start(out=out_d.ap(), in_=vall)

    nc.compile()
    return nc


def _host_weights(Wf1, bf1, Wf2, Wh1, bh1, Wh2, WT1, bT1, WT2,
                  Wtau1, btau1, Wtau2, Wpsi1, bpsi1, Wpsi2, WP):
    f = np.float64
    A = lambda a: np.asarray(a, f)
    Wf1, Wf2, Wh1, Wh2 = A(Wf1), A(Wf2), A(Wh1), A(Wh2)
    WT1, WT2, Wtau1, Wtau2 = A(WT1), A(WT2), A(Wtau1), A(Wtau2)
    Wpsi1, Wpsi2, WP = A(Wpsi1), A(Wpsi2), A(WP)
    Wpsi1z, Wpsi1y = Wpsi1[:, :NZ], Wpsi1[:, NZ:]

    vals = {
        "w_pre1": WT1, "w_pre2a": Wtau1 @ WT2, "w_tau1": Wtau1,
        "w_a3": WT1 @ Wtau2, "w_s": WP @ Wtau2, "w_tf": Wf1 @ Wtau2,
        "w_th": Wh1 @ Wtau2, "w_hd1": Wh1, "w_hd2n": -(Wh1 @ Wtau2),
        "w_ppsi1": Wpsi1z @ WT2, "w_ppsi2": Wpsi1y @ Wh2,
        "w_q3": Wpsi1z, "w_r": Wtau1 @ Wpsi2, "w_bk": Wh2.T @ Wh2 / R,
        "w_glin": Wh1.T, "w_f2p": Wf2, "w_tau2n": -Wtau2,
        "w_ones": np.ones((16, 16), f),
    }
    packs = {0: np.zeros((128, WCOLS0), np.float32),
             1: np.zeros((128, WCOLS1), np.float32)}
    for k, (p, in_p, out_p, c0, r0) in _WL.items():
        WT = vals[k].T          # per-lane lhsT block (in_p, out_p)
        assert WT.shape == (in_p, out_p), (k, WT.shape, (in_p, out_p))
        for L in range(LANES):
            packs[p][r0 + L * in_p:r0 + (L + 1) * in_p,
                     c0 + L * out_p:c0 + (L + 1) * out_p] = WT
    p0 = packs[0].astype(np_bf16)
    p1 = packs[1].astype(np_bf16)
    # biases: fp32 values bitcast into bf16 column pairs of pack0
    bvals = {"bT1": bT1, "btau1": btau1, "bh1": bh1, "bpsi1": bpsi1,
             "bf1": bf1, "zero": np.zeros(H, np.float32)}
    for i, n in enumerate(BIAS_NAMES):
        col = np.tile(np.asarray(bvals[n], np.float32), LANES).reshape(128, 1)
        p0[:, 2 * i:2 * i + 2] = col.view(np_bf16)
    return {"wcrit": p0, "wrest": p1}


_CACHE = {}


def _get_nc():
    if "nc" not in _CACHE:
        _CACHE["nc"] = build_nc()
    return _CACHE["nc"]


def _in_maps(x_batch, e_batch, wts):
    wmap = _host_weights(**wts)

    def lanes(a, rows):
        return (np.asarray(a, np.float32).reshape(LANES, COLS, rows)
                .transpose(0, 2, 1).reshape(LANES * rows, COLS))

    in_maps = []
    for c in range(NCORES):
        cs = slice(c * PER_CORE, (c + 1) * PER_CORE)
        xe = np.concatenate([lanes(x_batch[cs], N), lanes(e_batch[cs], NZ)],
                            axis=0)
        m = {"xe": np.ascontiguousarray(xe.astype(np_bf16))}
        m.update(wmap)
        in_maps.append(m)
    return in_maps


def _reduce(results):
    total = np.float64(0.0)
    for r in results:
        v = np.asarray(r["vout"], np.float64)      # (64, COLS)
        v = v.reshape(LANES, N, COLS)              # lane, feature, col
        total += np.sqrt((v * v).sum(axis=1)).sum()
    return np.asarray(total / B, dtype=np.float32)


def kernel(x_batch, e_batch, **wts):
    nc = _get_nc()
    in_maps = _in_maps(np.asarray(x_batch, np.float32),
                       np.asarray(e_batch, np.float32), wts)
    res = run_bass_kernel_spmd(nc, in_maps, core_ids=list(range(NCORES)))
    return _reduce(res.results)


if __name__ == "__main__":
    rng = np.random.default_rng(0)
    wts = {
        "Wf1": rng.normal(size=(H, N)) * .3, "bf1": rng.normal(size=(H,)) * .3,
        "Wf2": rng.normal(size=(N, H)) * .3,
        "Wh1": rng.normal(size=(H, N)) * .3, "bh1": rng.normal(size=(H,)) * .3,
        "Wh2": rng.normal(size=(OUT, H)) * .3,
        "WT1": rng.normal(size=(H, N)) * .3, "bT1": rng.normal(size=(H,)) * .3,
        "WT2": rng.normal(size=(NZ, H)) * .3,
        "Wtau1": rng.normal(size=(H, NZ)) * .3, "btau1": rng.normal(size=(H,)) * .3,
        "Wtau2": rng.normal(size=(N, H)) * .3,
        "Wpsi1": rng.normal(size=(H, NZ + OUT)) * .3, "bpsi1": rng.normal(size=(H,)) * .3,
        "Wpsi2": rng.normal(size=(NZ, H)) * .3,
        "WP": rng.normal(size=(N, N)) * .3,
    }
    x = rng.normal(size=(B, N)).astype(np.float32)
    e = (rng.normal(size=(B, NZ)) * 0.1).astype(np.float32)
    print(kernel(x, e, **{k: np.asarray(v, np.float32) for k, v in wts.items()}))
